# revision 1
# baseline (speedup 1.0000x reference)
"""LocallyConnected1d v5: consumption-ordered single-ring input stream.

Same math as v3/v4 (int8 weights + global scale folded into x, c128
matmuls, PE bias). Changes vs v4:
  * bias + x + weights all on the sync HWDGE ring, interleaved in the
    exact order the PE consumes them (per-queue FIFO packet draining
    guarantees delivery order, so x no longer starves the weight stream).
  * bias first -> the 8 upfront bias matmuls warm the PE ~3us earlier.
  * int8->bf16 dequant split DVE 84.6% / ACT 15.4% (both ~15.4us incl.
    ACT's 16 psum evictions).
  * small tail weight blocks (1 bank) to shorten the cast+MM tail.
  * per-bank output stores on the scalar ring.
"""

import numpy as np
import ml_dtypes

B = 128
C = 64
O = 64
L = 1024
KW = 7
PAD = 3
NCORES = 8
PC = L // NCORES
NJ = PC + 2 * PAD
NT = NJ // 2                # 67
NBANK = PC // 8             # 16
BANKW = 8 * O               # 512

TT_N = [2, 4, 6, 8, 6, 4, 2]
TT_LO = [max(0, 2 * tt - 6) for tt in range(7)]
TT_OFF = np.cumsum([0] + TT_N).tolist()
BANKC = 32 * O              # 2048 weight cols per bank
WCOLS = NBANK * BANKC       # 32768

BLOCKS = [1, 1, 2, 4, 4, 2, 1, 1]                # banks per weight DMA block
BLK_OF = np.cumsum([0] + BLOCKS).tolist()
XCHUNKS = [(0, 7), (7, 19), (19, 35), (35, 51), (51, 67)]  # tile ranges
# interleave: chunk i of x goes before weight block XPOS[i]
XPOS = [0, 1, 3, 4, 5]
DVE_FRAC = 0.85
NWARM = 9                                        # dummy warmup matmuls


def _quant(weight):
    wmax = float(np.abs(weight).max())
    sw = wmax / 127.0
    wq = np.clip(np.rint(weight / sw), -127, 127).astype(np.int8)
    return wq, sw


def _pack_weight(wq):
    dt = wq.dtype
    NG = NCORES * NBANK
    WP = np.zeros((NG, 2 * C, BANKC), dt)
    for tt in range(7):
        lo, n = TT_LO[tt], TT_N[tt]
        for h in range(2):
            for i in range(n):
                k = 2 * tt + h - (lo + i)
                if not (0 <= k < KW):
                    continue
                p0 = lo + i
                sl = wq[:, :, p0::8, k]          # (O, C, NG)
                c0 = (TT_OFF[tt] + i) * O
                WP[:, h * C:(h + 1) * C, c0:c0 + O] = sl.transpose(2, 1, 0)
    packs = []
    for m in range(NCORES):
        pm = WP[m * NBANK:(m + 1) * NBANK]
        packs.append(np.ascontiguousarray(
            pm.transpose(1, 0, 2).reshape(2 * C, WCOLS)))
    return packs


def _pack_x(x, sw):
    xp = np.zeros((B, C, L + 2 * PAD), np.float32)
    xp[:, :, PAD:PAD + L] = x * sw
    xt = np.ascontiguousarray(xp.transpose(1, 2, 0))
    packs = []
    for m in range(NCORES):
        s = xt[:, PC * m: PC * m + NJ, :]
        s = s.reshape(C, NT, 2, B).transpose(2, 0, 1, 3)
        packs.append(np.ascontiguousarray(
            s.reshape(2 * C, NT * B).astype(ml_dtypes.bfloat16)))
    return packs


def _pack_bias(bias):
    bt = np.ascontiguousarray(bias.T)
    packs = []
    for m in range(NCORES):
        bp = np.empty((1, PC * O + B), np.float32)
        bp[0, :PC * O] = bt[PC * m: PC * m + PC].reshape(-1)
        bp[0, PC * O:] = 1.0
        packs.append(bp.astype(ml_dtypes.bfloat16))
    return packs


def pack_all(x, weight, bias):
    wq, sw = _quant(weight)
    return _pack_x(x, sw), _pack_weight(wq), _pack_bias(bias)


def unpack_out(outs):
    full = [np.asarray(r, np.float32).reshape(B, PC, O).transpose(0, 2, 1)
            for r in outs]
    return np.ascontiguousarray(np.concatenate(full, axis=2))


_PROG = None


def _build_program():
    global _PROG
    if _PROG is not None:
        return _PROG

    import concourse.bacc as bacc
    import concourse.mybir as mybir
    import concourse.tile as tile

    F32 = mybir.dt.float32
    BF16 = mybir.dt.bfloat16
    I8 = mybir.dt.int8

    nc = bacc.Bacc("TRN2", target_bir_lowering=False, debug=False,
                   num_devices=NCORES)
    x_d = nc.dram_tensor("xp", (2 * C, NT * B), BF16, kind="ExternalInput")
    w_d = nc.dram_tensor("wp", (2 * C, WCOLS), I8, kind="ExternalInput")
    b_d = nc.dram_tensor("bp", (1, PC * O + B), BF16, kind="ExternalInput")
    o_d = nc.dram_tensor("out", (B, PC * O), BF16, kind="ExternalOutput")

    with tile.TileContext(nc) as tc:
        with (
            tc.tile_pool(name="xb", bufs=5) as xpool,
            tc.tile_pool(name="wi", bufs=5) as wipool,
            tc.tile_pool(name="wf", bufs=4) as wfpool,
            tc.tile_pool(name="cst", bufs=1) as cpool,
            tc.tile_pool(name="st", bufs=6) as spool,
            tc.tile_pool(name="ps", bufs=8, space="PSUM") as ppool,
        ):
            # PE warmup: memset-fed dummy matmuls need no DMA, so they can
            # start as soon as the engines come up -- the HAM clock gate is
            # released (~3.4us of sustained PE work) before real matmuls.
            wzs = cpool.tile([1, B], BF16)
            nc.vector.memset(wzs[:], 1.0)
            wzm = cpool.tile([1, BANKW], BF16)
            nc.vector.memset(wzm[:], 0.0)
            warm_ps = ppool.tile([B, BANKW], F32, tag="ps")
            for _ in range(NWARM):
                nc.tensor.matmul(warm_ps[:], wzs[0:1, :], wzm[0:1, :],
                                 start=True, stop=True)

            # sync ring, consumption order: bias, then x/w interleaved
            biast = cpool.tile([1, PC * O + B], BF16)
            nc.sync.dma_start(biast[:], b_d[:])
            ones = biast[0:1, PC * O: PC * O + B]

            x_tiles = []          # (t0, t1, tile)
            wi_tiles = []
            xi = 0
            for bi, nb in enumerate(BLOCKS):
                while xi < len(XCHUNKS) and XPOS[xi] == bi:
                    t0, t1 = XCHUNKS[xi]
                    xt = xpool.tile([2 * C, (t1 - t0) * B], BF16)
                    nc.sync.dma_start(xt[:], x_d[:, t0 * B:t1 * B])
                    x_tiles.append((t0, t1, xt))
                    xi += 1
                wt = wipool.tile([2 * C, nb * BANKC], I8)
                c0 = BLK_OF[bi] * BANKC
                nc.sync.dma_start(wt[:], w_d[:, c0:c0 + nb * BANKC])
                wi_tiles.append(wt)
            assert xi == len(XCHUNKS)

            def x_slice(t):
                for t0, t1, xt in x_tiles:
                    if t0 <= t < t1:
                        return xt[:, (t - t0) * B:(t - t0 + 1) * B]
                raise AssertionError(t)

            # dequant pieces (<=2 banks each), in bank order
            wf_tiles = {}
            for bi, nb in enumerate(BLOCKS):
                for p0 in range(0, nb, 2):
                    pb = min(2, nb - p0)         # banks in this piece
                    cols = pb * BANKC
                    src0 = (p0) * BANKC
                    wf = wfpool.tile([2 * C, cols], BF16)
                    dc = (int(cols * DVE_FRAC) // 64) * 64
                    nc.vector.tensor_copy(
                        wf[:, :dc], wi_tiles[bi][:, src0:src0 + dc])
                    nc.scalar.copy(
                        wf[:, dc:cols], wi_tiles[bi][:, src0 + dc:src0 + cols])
                    for j in range(pb):
                        wf_tiles[BLK_OF[bi] + p0 + j] = (wf, j)

            # upfront bias matmuls for banks 0-7 (PE warmup + group open)
            ps_tiles = {}
            for g in range(8):
                ps = ppool.tile([B, BANKW], F32, tag="ps")
                nc.tensor.matmul(
                    ps[:], ones, biast[0:1, BANKW * g: BANKW * (g + 1)],
                    start=True, stop=False)
                ps_tiles[g] = ps

            for g in range(NBANK):
                if g < 8:
                    ps = ps_tiles[g]
                else:
                    ps = ppool.tile([B, BANKW], F32, tag="ps")
                    nc.tensor.matmul(
                        ps[:], ones, biast[0:1, BANKW * g: BANKW * (g + 1)],
                        start=True, stop=False)
                for tt in range(7):
                    t = 4 * g + tt
                    lo, n = TT_LO[tt], TT_N[tt]
                    xs = x_slice(t)
                    wf, lb = wf_tiles[g]
                    wc = lb * BANKC + TT_OFF[tt] * O
                    ws = wf[:, wc: wc + n * O]
                    nc.tensor.matmul(
                        ps[:, lo * O:(lo + n) * O], xs, ws,
                        start=False, stop=(tt == 6))
                stage = spool.tile([B, BANKW], BF16)
                nc.scalar.copy(stage[:], ps[:])
                # stores ride the (otherwise idle) gpsimd SWDGE ring so the
                # ACT queue isn't clogged by ~600ns dma-issue instructions
                nc.gpsimd.dma_start(
                    o_d[:, BANKW * g: BANKW * (g + 1)], stage[:])

    nc.compile()
    _PROG = nc
    return nc


def _ensure_ntff_hook():
    import sys
    import types
    try:
        import antenv.axon_hooks  # noqa: F401
        return
    except ImportError:
        pass
    hook = None
    try:
        import contextlib
        import ctypes
        lib = ctypes.CDLL("/opt/axon/libaxon_pjrt.so")
        lib.axon_start_nrt_profile.argtypes = [
            ctypes.POINTER(ctypes.c_int64), ctypes.c_size_t]
        lib.axon_start_nrt_profile.restype = ctypes.c_int64
        lib.axon_stop_nrt_profile.argtypes = [ctypes.c_char_p]
        lib.axon_stop_nrt_profile.restype = ctypes.c_int64

        @contextlib.contextmanager
        def _hook(output_dir, device_ids):
            import jax
            jax.devices()
            if device_ids:
                ids = (ctypes.c_int64 * len(device_ids))(*device_ids)
                rc = lib.axon_start_nrt_profile(ids, len(device_ids))
            else:
                rc = lib.axon_start_nrt_profile(None, 0)
            if rc != 0:
                raise RuntimeError(f"axon_start_nrt_profile rc={rc}")
            try:
                yield
            finally:
                lib.axon_stop_nrt_profile(str(output_dir).encode())

        hook = _hook
    except Exception:
        hook = None
    mod = types.ModuleType("antenv.axon_hooks")
    mod.get_axon_ntff_profile_hook = lambda: hook
    mod.set_axon_ntff_profile_hook = lambda h: None
    try:
        import antenv
        antenv.axon_hooks = mod
    except ImportError:
        pass
    sys.modules["antenv.axon_hooks"] = mod


def _run(x, weight, bias, trace=False, tmpdir=None):
    from concourse.bass_utils import run_bass_kernel_spmd
    _ensure_ntff_hook()

    x = np.asarray(x, np.float32)
    weight = np.asarray(weight, np.float32)
    bias = np.asarray(bias, np.float32)
    xpacks, wpacks, bpacks = pack_all(x, weight, bias)
    nc = _build_program()
    in_maps = [{"xp": xpacks[m], "wp": wpacks[m], "bp": bpacks[m]}
               for m in range(NCORES)]
    res = run_bass_kernel_spmd(nc, in_maps, list(range(NCORES)), trace=trace,
                               tmpdir=tmpdir)
    full = unpack_out([r["out"] for r in res.results])
    return full, res


def kernel(x, weight, bias):
    out, _ = _run(x, weight, bias, trace=False)
    return out



# revision 2
# speedup vs baseline: 1.0505x; 1.0505x over previous
"""LocallyConnected1d v6: fp8e3m4 weights fed directly to the PE.

Key change vs v5: weights are quantized host-side to fp8 e3m4 (4 mantissa
bits, relmax ~8.5e-3 incl. bf16 x) and the matmuls consume them straight
from the DMA'd SBUF tiles as the moving operand -- the entire int8->bf16
dequant stage (~16us of DVE + ACT work that gated the PE and kept HAM at
1.2GHz until ~24us) is gone.  1/scale is folded into x like v5 folded the
int8 scale.  Evictions alternate DVE/ACT; per-bank stores stay on the
gpsimd SWDGE ring.
"""

import numpy as np
import ml_dtypes

B = 128
C = 64
O = 64
L = 1024
KW = 7
PAD = 3
NCORES = 8
PC = L // NCORES
NJ = PC + 2 * PAD
NT = NJ // 2                # 67
NBANK = PC // 8             # 16
BANKW = 8 * O               # 512

TT_N = [2, 4, 6, 8, 6, 4, 2]
TT_LO = [max(0, 2 * tt - 6) for tt in range(7)]
TT_OFF = np.cumsum([0] + TT_N).tolist()
BANKC = 32 * O              # 2048 weight cols per bank
WCOLS = NBANK * BANKC       # 32768

BLOCKS = [1, 1, 2, 4, 4, 2, 1, 1]                # banks per weight DMA block
BLK_OF = np.cumsum([0] + BLOCKS).tolist()
XCHUNKS = [(0, 7), (7, 19), (19, 35), (35, 51), (51, 67)]  # tile ranges
# interleave: chunk i of x goes before weight block XPOS[i]
XPOS = [0, 1, 3, 4, 5]
NWARM = 8                                        # dummy warmup matmuls
WMAX_TGT = 15.5                                  # e3m4 max normal


def _quant(weight):
    wmax = float(np.abs(weight).max())
    sw = WMAX_TGT / wmax
    wq = (weight * sw).astype(ml_dtypes.float8_e3m4)
    return wq, sw


def _pack_weight(wq):
    dt = wq.dtype
    NG = NCORES * NBANK
    WP = np.zeros((NG, 2 * C, BANKC), dt)
    for tt in range(7):
        lo, n = TT_LO[tt], TT_N[tt]
        for h in range(2):
            for i in range(n):
                k = 2 * tt + h - (lo + i)
                if not (0 <= k < KW):
                    continue
                p0 = lo + i
                sl = wq[:, :, p0::8, k]          # (O, C, NG)
                c0 = (TT_OFF[tt] + i) * O
                WP[:, h * C:(h + 1) * C, c0:c0 + O] = sl.transpose(2, 1, 0)
    packs = []
    for m in range(NCORES):
        pm = WP[m * NBANK:(m + 1) * NBANK]
        packs.append(np.ascontiguousarray(
            pm.transpose(1, 0, 2).reshape(2 * C, WCOLS)))
    return packs


def _pack_x(x, inv_sw):
    xp = np.zeros((B, C, L + 2 * PAD), np.float32)
    xp[:, :, PAD:PAD + L] = x * inv_sw
    xt = np.ascontiguousarray(xp.transpose(1, 2, 0))
    packs = []
    for m in range(NCORES):
        s = xt[:, PC * m: PC * m + NJ, :]
        s = s.reshape(C, NT, 2, B).transpose(2, 0, 1, 3)
        packs.append(np.ascontiguousarray(
            s.reshape(2 * C, NT * B).astype(ml_dtypes.bfloat16)))
    return packs


def _pack_bias(bias):
    bt = np.ascontiguousarray(bias.T)
    packs = []
    for m in range(NCORES):
        bp = np.empty((1, PC * O + B), np.float32)
        bp[0, :PC * O] = bt[PC * m: PC * m + PC].reshape(-1)
        bp[0, PC * O:] = 1.0
        packs.append(bp.astype(ml_dtypes.bfloat16))
    return packs


def pack_all(x, weight, bias):
    wq, sw = _quant(weight)
    return _pack_x(x, 1.0 / sw), _pack_weight(wq), _pack_bias(bias)


def unpack_out(outs):
    full = [np.asarray(r, np.float32).reshape(B, PC, O).transpose(0, 2, 1)
            for r in outs]
    return np.ascontiguousarray(np.concatenate(full, axis=2))


_PROG = None


def _build_program():
    global _PROG
    if _PROG is not None:
        return _PROG

    import concourse.bacc as bacc
    import concourse.mybir as mybir
    import concourse.tile as tile

    F32 = mybir.dt.float32
    BF16 = mybir.dt.bfloat16
    F8E3 = mybir.dt.float8e3

    nc = bacc.Bacc("TRN2", target_bir_lowering=False, debug=False,
                   num_devices=NCORES)
    x_d = nc.dram_tensor("xp", (2 * C, NT * B), BF16, kind="ExternalInput")
    w_d = nc.dram_tensor("wp", (2 * C, WCOLS), F8E3, kind="ExternalInput")
    b_d = nc.dram_tensor("bp", (1, PC * O + B), BF16, kind="ExternalInput")
    o_d = nc.dram_tensor("out", (B, PC * O), BF16, kind="ExternalOutput")

    with tile.TileContext(nc) as tc:
        with (
            tc.tile_pool(name="xb", bufs=5) as xpool,
            tc.tile_pool(name="wi", bufs=8) as wipool,
            tc.tile_pool(name="cst", bufs=1) as cpool,
            tc.tile_pool(name="st", bufs=6) as spool,
            tc.tile_pool(name="ps", bufs=8, space="PSUM") as ppool,
        ):
            # PE warmup: memset-fed dummy matmuls need no DMA, so they run
            # while the first real DMAs are still in flight; keeps the HAM
            # activity window filling from the moment the queue unblocks.
            wzs = cpool.tile([1, B], BF16)
            nc.vector.memset(wzs[:], 1.0)
            wzm = cpool.tile([1, B], BF16)
            nc.vector.memset(wzm[:], 0.0)
            warm_ps = ppool.tile([B, BANKW], F32, tag="ps")
            for _ in range(NWARM):
                nc.tensor.matmul(warm_ps[:, :B], wzs[0:1, :], wzm[0:1, :],
                                 start=True, stop=True)

            # sync ring, consumption order: bias, then x/w interleaved
            biast = cpool.tile([1, PC * O + B], BF16)
            nc.sync.dma_start(biast[:], b_d[:])
            ones = biast[0:1, PC * O: PC * O + B]

            x_tiles = []          # (t0, t1, tile)
            wi_tiles = []
            xi = 0
            for bi, nb in enumerate(BLOCKS):
                while xi < len(XCHUNKS) and XPOS[xi] == bi:
                    t0, t1 = XCHUNKS[xi]
                    xt = xpool.tile([2 * C, (t1 - t0) * B], BF16)
                    nc.sync.dma_start(xt[:], x_d[:, t0 * B:t1 * B])
                    x_tiles.append((t0, t1, xt))
                    xi += 1
                wt = wipool.tile([2 * C, nb * BANKC], F8E3)
                c0 = BLK_OF[bi] * BANKC
                nc.sync.dma_start(wt[:], w_d[:, c0:c0 + nb * BANKC])
                wi_tiles.append(wt)
            assert xi == len(XCHUNKS)

            def x_slice(t):
                for t0, t1, xt in x_tiles:
                    if t0 <= t < t1:
                        return xt[:, (t - t0) * B:(t - t0 + 1) * B]
                raise AssertionError(t)

            def w_slice(g, cols_lo, cols_hi):
                # bank g lives in block bi at local bank lb
                for bi, nb in enumerate(BLOCKS):
                    if BLK_OF[bi] <= g < BLK_OF[bi + 1]:
                        lb = g - BLK_OF[bi]
                        base = lb * BANKC
                        return wi_tiles[bi][:, base + cols_lo: base + cols_hi]
                raise AssertionError(g)

            for g in range(NBANK):
                ps = ppool.tile([B, BANKW], F32, tag="ps")
                nc.tensor.matmul(
                    ps[:], ones, biast[0:1, BANKW * g: BANKW * (g + 1)],
                    start=True, stop=False)
                for tt in range(7):
                    t = 4 * g + tt
                    lo, n = TT_LO[tt], TT_N[tt]
                    xs = x_slice(t)
                    wc = TT_OFF[tt] * O
                    ws = w_slice(g, wc, wc + n * O)
                    nc.tensor.matmul(
                        ps[:, lo * O:(lo + n) * O], xs, ws,
                        start=False, stop=(tt == 6))
                stage = spool.tile([B, BANKW], BF16)
                if g % 2 == 0:
                    nc.vector.tensor_copy(stage[:], ps[:])
                else:
                    nc.scalar.copy(stage[:], ps[:])
                # stores ride the (otherwise idle) gpsimd SWDGE ring
                nc.gpsimd.dma_start(
                    o_d[:, BANKW * g: BANKW * (g + 1)], stage[:])

    nc.compile()
    _PROG = nc
    return nc


def _ensure_ntff_hook():
    import sys
    import types
    try:
        import antenv.axon_hooks  # noqa: F401
        return
    except ImportError:
        pass
    hook = None
    try:
        import contextlib
        import ctypes
        lib = ctypes.CDLL("/opt/axon/libaxon_pjrt.so")
        lib.axon_start_nrt_profile.argtypes = [
            ctypes.POINTER(ctypes.c_int64), ctypes.c_size_t]
        lib.axon_start_nrt_profile.restype = ctypes.c_int64
        lib.axon_stop_nrt_profile.argtypes = [ctypes.c_char_p]
        lib.axon_stop_nrt_profile.restype = ctypes.c_int64

        @contextlib.contextmanager
        def _hook(output_dir, device_ids):
            import jax
            jax.devices()
            if device_ids:
                ids = (ctypes.c_int64 * len(device_ids))(*device_ids)
                rc = lib.axon_start_nrt_profile(ids, len(device_ids))
            else:
                rc = lib.axon_start_nrt_profile(None, 0)
            if rc != 0:
                raise RuntimeError(f"axon_start_nrt_profile rc={rc}")
            try:
                yield
            finally:
                lib.axon_stop_nrt_profile(str(output_dir).encode())

        hook = _hook
    except Exception:
        hook = None
    mod = types.ModuleType("antenv.axon_hooks")
    mod.get_axon_ntff_profile_hook = lambda: hook
    mod.set_axon_ntff_profile_hook = lambda h: None
    try:
        import antenv
        antenv.axon_hooks = mod
    except ImportError:
        pass
    sys.modules["antenv.axon_hooks"] = mod


def _run(x, weight, bias, trace=False, tmpdir=None):
    from concourse.bass_utils import run_bass_kernel_spmd
    _ensure_ntff_hook()

    x = np.asarray(x, np.float32)
    weight = np.asarray(weight, np.float32)
    bias = np.asarray(bias, np.float32)
    xpacks, wpacks, bpacks = pack_all(x, weight, bias)
    nc = _build_program()
    in_maps = [{"xp": xpacks[m], "wp": wpacks[m], "bp": bpacks[m]}
               for m in range(NCORES)]
    res = run_bass_kernel_spmd(nc, in_maps, list(range(NCORES)), trace=trace,
                               tmpdir=tmpdir)
    full = unpack_out([r["out"] for r in res.results])
    return full, res


def kernel(x, weight, bias):
    out, _ = _run(x, weight, bias, trace=False)
    return out
